# revision 6
# baseline (speedup 1.0000x reference)
"""Bass/Tile TRN2 kernel for nn_Actor_DeepSet (8-core data parallel).

Reference computation (per row r = b*8 + i, obs=64, hidden=128):
  h1   = relu(x_r @ w1.T + b1)
  hsum = (1/8) * sum_{k=1..7} relu(rot_{i+1}(x_{b,k}) @ w1o.T + b1o)
  h2   = relu([h1, hsum] @ w2.T + b2)
  out  = h2 @ wv.T + bv
rot_s rotates the 64 features; equivalently a column rotation of w1o.  The
1/8 folds into w1o/b1o (relu positively homogeneous).

Device layout: transposed (channels on partitions, rows on free axis), bf16
in / f32 PSUM.  Each 512-row tile is reordered agent-major on the host:
tile column j = a*64 + b, so the "other agents" moving operand is columns
64..511 (contiguous).  Layer-1 "other" products land k-major in PSUM; relu'd
slabs are stored [128, k=7, s=8, b=64] in SBUF; the k-sum runs partly as DVE
adds and partly folded into layer-2 PSUM accumulation.  Biases are applied by
the drain ops (ACT bias / DVE tensor_scalar add+max).  Tiles are processed in
pairs with matmuls grouped by stationary operand to share LDWEIGHTS.
Output y.T [16, 16384] in tile-(a,b) order; host unscrambles.
"""

import os
import numpy as np

import concourse.bacc as bacc
import concourse.mybir as mybir
import concourse.tile as tile
from concourse.bass_utils import run_bass_kernel_spmd

N_CORES = 8
N_AGENTS = 8
OBS = 64
HIDDEN = 128
NUM_OUT = 16
ROWS_PC = 16384            # rows per core
TILE_N = 512               # rows per tile
N_TILES = ROWS_PC // TILE_N
NB = TILE_N // N_AGENTS    # batches per tile (64)

# tuning knobs
N_FOLD = int(os.environ.get("KN_FOLD", "2"))        # k-slabs folded into L2 PSUM
N_DVE_SHIFTS = int(os.environ.get("KN_DVE_SHIFTS", "2"))  # shift drains on DVE
O_ON_ACT = bool(int(os.environ.get("KN_O_ACT", "1")))
HTOP_ON_ACT = bool(int(os.environ.get("KN_HTOP_ACT", "0")))

BF16 = mybir.dt.bfloat16
F32 = mybir.dt.float32
NP_BF16 = mybir.dt.np(BF16)
ALU = mybir.AluOpType
AF = mybir.ActivationFunctionType

_compiled_nc = None
last_exec_time_ns = None


def _build_nc():
    nc = bacc.Bacc("TRN2", target_bir_lowering=False, debug=False,
                   num_devices=N_CORES)

    x_ext = nc.dram_tensor("x", [OBS, ROWS_PC], BF16, kind="ExternalInput")
    wl1_ext = nc.dram_tensor("wl1", [OBS, HIDDEN], BF16, kind="ExternalInput")
    wcat_ext = nc.dram_tensor("wcat", [N_AGENTS, OBS, HIDDEN], BF16,
                              kind="ExternalInput")
    w2a_ext = nc.dram_tensor("w2a", [HIDDEN, HIDDEN], BF16, kind="ExternalInput")
    w2b_ext = nc.dram_tensor("w2b", [HIDDEN, HIDDEN], BF16, kind="ExternalInput")
    wv_ext = nc.dram_tensor("wv", [HIDDEN, NUM_OUT], BF16, kind="ExternalInput")
    b1_ext = nc.dram_tensor("b1", [HIDDEN, 1], F32, kind="ExternalInput")
    b1o_ext = nc.dram_tensor("b1o", [HIDDEN, 1], F32, kind="ExternalInput")
    b2_ext = nc.dram_tensor("b2", [HIDDEN, 1], F32, kind="ExternalInput")
    bv_ext = nc.dram_tensor("bv", [NUM_OUT, 1], F32, kind="ExternalInput")
    y_ext = nc.dram_tensor("y", [NUM_OUT, ROWS_PC], F32, kind="ExternalOutput")

    with tile.TileContext(nc) as tc:
        with (
            tc.tile_pool(name="const", bufs=1) as cpool,
            tc.tile_pool(name="xin", bufs=4) as xpool,
            tc.tile_pool(name="act", bufs=4) as apool,
            tc.tile_pool(name="rbuf", bufs=3) as rpool,
            tc.tile_pool(name="outb", bufs=4) as opool,
            tc.tile_pool(name="ps_mm", bufs=3, space="PSUM") as pmm,
            tc.tile_pool(name="ps_o", bufs=1, space="PSUM") as pso,
            tc.tile_pool(name="ps_s", bufs=4, space="PSUM") as pss,
        ):
            # --- persistent weights / biases ---
            wl1 = cpool.tile([OBS, HIDDEN], BF16)
            nc.sync.dma_start(wl1[:], wl1_ext[:])
            wcat = cpool.tile([OBS, N_AGENTS * HIDDEN], BF16)
            for s in range(N_AGENTS):
                nc.sync.dma_start(wcat[:, s * HIDDEN:(s + 1) * HIDDEN],
                                  wcat_ext[s])
            w2a = cpool.tile([HIDDEN, HIDDEN], BF16)
            nc.sync.dma_start(w2a[:], w2a_ext[:])
            w2b = cpool.tile([HIDDEN, HIDDEN], BF16)
            nc.sync.dma_start(w2b[:], w2b_ext[:])
            wv = cpool.tile([HIDDEN, NUM_OUT], BF16)
            nc.sync.dma_start(wv[:], wv_ext[:])
            b1t = cpool.tile([HIDDEN, 1], F32)
            nc.sync.dma_start(b1t[:], b1_ext[:])
            b1ot = cpool.tile([HIDDEN, 1], F32)
            nc.sync.dma_start(b1ot[:], b1o_ext[:])
            b2t = cpool.tile([HIDDEN, 1], F32)
            nc.sync.dma_start(b2t[:], b2_ext[:])
            bvt = cpool.tile([NUM_OUT, 1], F32)
            nc.sync.dma_start(bvt[:], bv_ext[:])

            n_tt = 6 - N_FOLD

            def drain(dst, src, bias, on_act):
                """relu(src + bias) -> dst from PSUM."""
                if on_act:
                    nc.scalar.activation(dst, src, AF.Relu, bias=bias)
                else:
                    nc.vector.tensor_scalar(dst, src, bias, 0.0,
                                            ALU.add, ALU.max)

            for pair in range(N_TILES // 2):
                tiles = (2 * pair, 2 * pair + 1)
                xts, htops, rs, rks, hbots, ps2s, h2s = {}, {}, {}, {}, {}, {}, {}

                for t in tiles:
                    xt = xpool.tile([OBS, TILE_N], BF16)
                    nc.sync.dma_start(xt[:],
                                      x_ext[:, t * TILE_N:(t + 1) * TILE_N])
                    xts[t] = xt

                # layer-1 self: shared wl1 stationary
                ps1s = {}
                for t in tiles:
                    ps1 = pmm.tile([HIDDEN, TILE_N], F32, tag="mm")
                    nc.tensor.matmul(ps1[:], wl1[:], xts[t][:])
                    ps1s[t] = ps1
                for t in tiles:
                    htop = apool.tile([HIDDEN, TILE_N], BF16, tag="htop")
                    drain(htop[:], ps1s[t][:], b1t[:], HTOP_ON_ACT)
                    htops[t] = htop

                # layer-1 others: 8 shifts x 2 tiles, shared stationary per s
                for t in tiles:
                    r = rpool.tile([HIDDEN, 7 * N_AGENTS * NB], BF16)
                    rs[t] = r
                    rks[t] = r[:].rearrange("p (k c) -> p k c", k=7)
                for s in range(N_AGENTS):
                    w_s = wcat[:, s * HIDDEN:(s + 1) * HIDDEN]
                    for t in tiles:
                        ps = pss.tile([HIDDEN, 7 * NB], F32)
                        nc.tensor.matmul(ps[:], w_s, xts[t][:, NB:TILE_N])
                        r_v = rs[t][:].rearrange("p (k s b) -> p k s b",
                                                 k=7, s=N_AGENTS)
                        drain(r_v[:, :, s, :],
                              ps[:].rearrange("p (k b) -> p k b", k=7),
                              b1ot[:], s >= N_DVE_SHIFTS)

                # partial k-sums on DVE
                for t in tiles:
                    hbot = apool.tile([HIDDEN, N_AGENTS * NB], BF16, tag="hbot")
                    r_k = rks[t]
                    with nc.allow_low_precision("bf16 partial sums"):
                        if n_tt == 0:
                            hbot = None
                        else:
                            nc.vector.tensor_add(hbot[:], r_k[:, 0, :],
                                                 r_k[:, 1, :])
                            for k in range(2, n_tt + 1):
                                nc.vector.tensor_add(hbot[:], hbot[:],
                                                     r_k[:, k, :])
                    hbots[t] = hbot

                # layer 2: w2a group then w2b group (stationary reuse)
                for t in tiles:
                    ps2 = pmm.tile([HIDDEN, TILE_N], F32, tag="mm")
                    nc.tensor.matmul(ps2[:], w2a[:], htops[t][:],
                                     start=True, stop=False)
                    ps2s[t] = ps2
                first_fold = 7 - N_FOLD if n_tt > 0 else 0
                for t in tiles:
                    if hbots[t] is not None:
                        nc.tensor.matmul(ps2s[t][:], w2b[:], hbots[t][:],
                                         start=False, stop=(N_FOLD == 0))
                    for k in range(first_fold, 7):
                        nc.tensor.matmul(ps2s[t][:], w2b[:], rks[t][:, k, :],
                                         start=False, stop=(k == 6))
                for t in tiles:
                    h2 = apool.tile([HIDDEN, TILE_N], BF16, tag="h2")
                    nc.vector.tensor_scalar(h2[:], ps2s[t][:], b2t[:], 0.0,
                                            ALU.add, ALU.max)
                    h2s[t] = h2

                # layer 3: shared wv stationary
                ps3s = {}
                for t in tiles:
                    ps3 = pso.tile([NUM_OUT, TILE_N], F32)
                    nc.tensor.matmul(ps3[:], wv[:], h2s[t][:])
                    ps3s[t] = ps3
                for t in tiles:
                    o = opool.tile([NUM_OUT, TILE_N], F32)
                    if O_ON_ACT:
                        nc.scalar.activation(o[:], ps3s[t][:], AF.Identity,
                                             bias=bvt[:])
                    else:
                        nc.vector.tensor_scalar_add(o[:], ps3s[t][:], bvt[:])
                    nc.sync.dma_start(y_ext[:, t * TILE_N:(t + 1) * TILE_N],
                                      o[:])

    nc.compile()
    return nc


def kernel(inputs, w1, b1, w1o, b1o, w2, b2, wv, bv):
    global _compiled_nc, last_exec_time_ns
    if _compiled_nc is None:
        _compiled_nc = _build_nc()
    nc = _compiled_nc

    inputs = np.asarray(inputs, dtype=np.float32)
    w1 = np.asarray(w1, dtype=np.float32)
    b1 = np.asarray(b1, dtype=np.float32)
    w1o = np.asarray(w1o, dtype=np.float32)
    b1o = np.asarray(b1o, dtype=np.float32)
    w2 = np.asarray(w2, dtype=np.float32)
    b2 = np.asarray(b2, dtype=np.float32)
    wv = np.asarray(wv, dtype=np.float32)
    bv = np.asarray(bv, dtype=np.float32)

    # host-side weight prep (tiny)
    wl1 = np.ascontiguousarray(w1.T).astype(NP_BF16)          # [64, 128]
    wcat = np.empty((N_AGENTS, OBS, HIDDEN), dtype=NP_BF16)
    for si in range(N_AGENTS):
        s = si + 1  # shift amount for agent i = si
        wcat[si] = (np.roll(w1o, s, axis=1).T / N_AGENTS).astype(NP_BF16)
    w2a = np.ascontiguousarray(w2[:, :HIDDEN].T).astype(NP_BF16)
    w2b = np.ascontiguousarray(w2[:, HIDDEN:].T).astype(NP_BF16)
    wvt = np.ascontiguousarray(wv.T).astype(NP_BF16)
    b1c = np.ascontiguousarray(b1[:, None]).astype(np.float32)
    b1oc = np.ascontiguousarray((b1o / N_AGENTS)[:, None]).astype(np.float32)
    b2c = np.ascontiguousarray(b2[:, None]).astype(np.float32)
    bvc = np.ascontiguousarray(bv[:, None]).astype(np.float32)

    # shard rows across cores; x.T columns reordered per tile to (a, b)
    xs = inputs.reshape(N_CORES, N_TILES, NB, N_AGENTS, OBS)
    xs_t = xs.transpose(0, 4, 1, 3, 2).reshape(N_CORES, OBS, ROWS_PC)
    in_maps = []
    for c in range(N_CORES):
        in_maps.append({
            "x": np.ascontiguousarray(xs_t[c]).astype(NP_BF16),
            "wl1": wl1, "wcat": wcat, "w2a": w2a, "w2b": w2b, "wv": wvt,
            "b1": b1c, "b1o": b1oc, "b2": b2c, "bv": bvc,
        })

    trace = bool(int(os.environ.get("BASS_KERNEL_TRACE", "0")))
    res = run_bass_kernel_spmd(nc, in_maps, list(range(N_CORES)), trace=trace)
    last_exec_time_ns = res.exec_time_ns

    y = np.stack([res.results[c]["y"] for c in range(N_CORES)])  # [8,16,16384]
    # y columns are (tile, agent, batch); rows are (tile, batch, agent)
    y = y.reshape(N_CORES, NUM_OUT, N_TILES, N_AGENTS, NB)
    out = y.transpose(0, 2, 4, 3, 1).reshape(N_CORES * ROWS_PC, NUM_OUT)
    return np.ascontiguousarray(out, dtype=np.float32)
